# revision 45
# baseline (speedup 1.0000x reference)
"""Multi-head attention (B=4, S=2048, H=1024, NH=16, D=64) on 8 trn2 cores.

Sharding: core c = (batch b = c//2, head-group g = c%2); each core owns 8
heads of one batch (tensor-parallel split of Wq/Wk/Wv columns + Wo rows,
data-parallel over batch). Each core computes the partial output
contribution yT_partial = Wo_g @ attn_g(b).T in transposed layout; the
host sums the two head-group partials per batch (the standard unshard for
a row-sharded Wo) and transposes back.

Device dataflow (all bf16 matmuls, fp32 PSUM accumulation):
  - host pre-transposes x and the weight slices so no on-device transpose
    of activations is ever needed:
      xT (H, S), wqT/wkT/wvT (H, HG), woT (HG, H)
  - qT = WqT.T-contracted projection -> (HG, S) transposed layout; same kT
  - v in natural layout (S, HG), stored as per-128-row j-tiles with a
    ones-column appended per head (the softmax denominator trick)
  - per (head, 1024-query block, 128-key tile j):
      scoresT_j = kT_h_j.T @ qT_h          [128 keys x 1024 queries] PSUM
      P_j = exp(scoresT_j / 8)             ScalarE, PSUM->SBUF bf16
      av += [v_j | 1 | pad].T @ P_j        [128 x 1024] PSUM (row 64 = denom)
    softmax skips the max-subtraction: q,k ~ N(0,1) so scores/8 ~ N(0,1),
    max |score| ~ 5.5 -> exp is far from overflow; result is identical to
    the stable softmax up to fp rounding.
  - normalize: denom row scattered to 64 lanes by DMA, DVE reciprocal
    (iterative 8 cyc/elem -- on one lane it would cost 6.5us), gathered
    back, broadcast across partitions via a K=1 f32 matmul against a ones
    row (the GPSIMD partition_broadcast op produces garbage on HW), then
    DVE multiply -> attn_sb transposed (HG, S) bf16.  Odd heads land on
    lanes 64..127 via an SBUF->SBUF DMA (DVE cannot cross partitions).
    Normalization of block i and its last AV matmul + PSUM eviction are
    deferred into block i+1's j-loop so block boundaries never stall the
    exp stream.
  - yT = woT-contracted projection + bias -> (H, S) f32 -> DRAM.

HAM clock-gate discipline (the big perf lever on trn2): K=64 or M<128
matmuls on a fixed row-half read as "low activity" and lock the PE at
half clock even when issued back-to-back.  Scores therefore run their two
query-halves on complementary row-groups (via swapped-half duplicates of
qT/kT), projections split their contraction into even/odd 64-row halves
in two PSUM banks, and the AV stationary operand is widened to the full
128 columns.

Biases: bq/bk are applied on-device (per-partition bias in transposed
layout). bv folds algebraically into the output bias: softmax rows sum to
1, so attn(v + 1*bv.T) = attn(v) + bv, hence y += bv @ Wo.T; the host
computes bo_eff = bo + bv @ Wo.T and the g==0 core adds it (g==1 gets
zeros so the host-side sum adds it exactly once).
"""

import sys

if "/opt/trn_rl_repo" not in sys.path:
    sys.path.insert(0, "/opt/trn_rl_repo")

import ml_dtypes
import numpy as np

import concourse.bass as bass
import concourse.tile as tile
from concourse import bacc, mybir
from concourse.bass_utils import run_bass_kernel_spmd

BF = mybir.dt.bfloat16
F32 = mybir.dt.float32
AF = mybir.ActivationFunctionType

B, S, H = 4, 2048, 1024
NH, D = 16, 64
HG = H // 2          # head-group width (8 heads)
NHC = 8              # heads per core
KT = H // 128        # k-tiles over H contraction
CT = HG // 128       # tiles over head-group dims
JT = S // 128        # key tiles
QW = 1024            # query-block width
NQB = S // QW
SCALE = 0.125        # 1/sqrt(D)

_CACHE = {}
LAST_RESULT = None


def _build():
    if "nc" in _CACHE:
        return _CACHE["nc"]

    nc = bacc.Bacc("TRN2", target_bir_lowering=False, debug=False, num_devices=8)

    xT_d = nc.dram_tensor("xT", [H, S], BF, kind="ExternalInput")
    wq_d = nc.dram_tensor("wq", [H, HG], BF, kind="ExternalInput")
    wk_d = nc.dram_tensor("wk", [H, HG], BF, kind="ExternalInput")
    wv_d = nc.dram_tensor("wv", [H, HG], BF, kind="ExternalInput")
    wo_d = nc.dram_tensor("wo", [HG, H], BF, kind="ExternalInput")
    bq_d = nc.dram_tensor("bq", [128, CT], F32, kind="ExternalInput")
    bk_d = nc.dram_tensor("bk", [128, CT], F32, kind="ExternalInput")
    bo_d = nc.dram_tensor("bo", [128, KT], F32, kind="ExternalInput")
    yT_d = nc.dram_tensor("yT", [H, S], F32, kind="ExternalOutput")

    with tile.TileContext(nc) as tc:
        with (
            tc.tile_pool(name="const", bufs=1) as constp,
            tc.tile_pool(name="ex_ps", bufs=2, space=bass.MemorySpace.PSUM) as ex_ps,
            tc.tile_pool(name="sc_ps", bufs=2, space=bass.MemorySpace.PSUM) as sc_ps,
            tc.tile_pool(name="av_ps", bufs=1, space=bass.MemorySpace.PSUM) as av_ps,
            tc.tile_pool(name="pp", bufs=4) as pp,
            tc.tile_pool(name="rcp", bufs=1) as rcp,
            tc.tile_pool(name="asb", bufs=2) as asbp,
            tc.tile_pool(name="tmpp", bufs=2) as tmpp,
            tc.tile_pool(name="yp", bufs=2) as yp,
        ):
            xTs = constp.tile([128, KT, S], BF)
            wqs = constp.tile([128, KT, HG], BF)
            wks = constp.tile([128, KT, HG], BF)
            wvs = constp.tile([128, KT, HG], BF)
            wos = constp.tile([128, CT, H], BF)
            bqs = constp.tile([128, CT], F32)
            bks = constp.tile([128, CT], F32)
            bos = constp.tile([128, KT], F32)
            ones = constp.tile([65, 64], F32)
            warm = constp.tile([1, 8], F32)
            qTs = constp.tile([128, CT, S], BF)
            kTs = constp.tile([128, CT, S], BF)
            # Swapped-half duplicates (partitions 0:64 <-> 64:128): the two
            # query-halves of each scores tile run on complementary PE
            # row-groups, overlapping their LDWEIGHTS and keeping the HAM
            # clock-gate warm (K=64 on a fixed half locks the PE cold).
            qTs2 = constp.tile([128, CT, S], BF)
            kTs2 = constp.tile([128, CT, S], BF)
            # Per j-tile: 8 head-blocks of [64 v-dims | ones]; 64 columns of
            # padding at the end so every head's AV lhsT can be widened to
            # 128 columns (head h reads cols h*65 .. h*65+128 — the extra 63
            # columns produce discarded rows but keep the PE array at full
            # column activity, which the HAM clock-gate needs to stay warm).
            vs = constp.tile([128, JT, NHC * 65 + 64], BF)
            attns = constp.tile([128, CT, S], BF)

            # Preload the ACT exp table while input DMAs run.
            nc.vector.memset(warm[:], 0.0)
            nc.scalar.activation(warm[:], warm[:], AF.Exp)

            nc.sync.dma_start(bqs[:], bq_d[:])
            nc.sync.dma_start(bks[:], bk_d[:])
            nc.sync.dma_start(bos[:], bo_d[:])
            for k in range(KT):
                nc.sync.dma_start(wqs[:, k, :], wq_d[k * 128:(k + 1) * 128, :])
                nc.sync.dma_start(wks[:, k, :], wk_d[k * 128:(k + 1) * 128, :])
                nc.sync.dma_start(
                    xTs[:, k, 0:S // 2], xT_d[k * 128:(k + 1) * 128, 0:S // 2]
                )
            for k in range(KT):
                nc.sync.dma_start(
                    xTs[:, k, S // 2:], xT_d[k * 128:(k + 1) * 128, S // 2:]
                )
            for k in range(KT):
                nc.sync.dma_start(wvs[:, k, :], wv_d[k * 128:(k + 1) * 128, :])
            for c in range(CT):
                nc.sync.dma_start(wos[:, c, :], wo_d[c * 128:(c + 1) * 128, :])

            nc.vector.memset(
                vs[:, :, 0:NHC * 65].rearrange("p j (h c) -> p j h c", c=65)[:, :, :, 64],
                1.0,
            )
            nc.vector.memset(vs[:, :, NHC * 65:], 1.0)
            nc.vector.memset(ones[:], 1.0)

            # All projections split the 128-row contraction tiles into
            # even/odd 64-row halves accumulating in two 1-bank PSUM tiles:
            # consecutive matmuls alternate PE row-groups, so each
            # LDWEIGHTS overlaps the in-flight matmul instead of
            # serializing (~107 ns/MM saved).  A DVE pass combines the
            # halves (via an f32 staging tile to avoid double rounding).
            def proj_qk(m, n, ws, dst, dst2, bias):
                msl = slice(m * 128, (m + 1) * 128)
                nsl = slice(n * 512, (n + 1) * 512)
                psA = ex_ps.tile([128, 512], F32, tag="ex")
                psB = ex_ps.tile([128, 512], F32, tag="ex")
                for k in range(KT):
                    nc.tensor.matmul(
                        psA[:], ws[0:64, k, msl], xTs[0:64, k, nsl],
                        start=(k == 0), stop=(k == KT - 1),
                    )
                    nc.tensor.matmul(
                        psB[:], ws[64:128, k, msl], xTs[64:128, k, nsl],
                        start=(k == 0), stop=(k == KT - 1),
                    )
                st = yp.tile([128, 512], F32, tag="y")
                nc.vector.tensor_scalar_add(st[:], psA[:], bias[:, m:m + 1])
                nc.vector.tensor_add(dst[:, m, nsl], psB[:], st[:])
                nc.gpsimd.dma_start(dst2[64:128, m, nsl], dst[0:64, m, nsl])
                nc.gpsimd.dma_start(dst2[0:64, m, nsl], dst[64:128, m, nsl])

            def proj_v(j):
                jsl = slice(j * 128, (j + 1) * 128)
                psA = ex_ps.tile([128, 512], F32, tag="ex")
                psB = ex_ps.tile([128, 512], F32, tag="ex")
                for k in range(KT):
                    nc.tensor.matmul(
                        psA[:], xTs[0:64, k, jsl], wvs[0:64, k, :],
                        start=(k == 0), stop=(k == KT - 1),
                    )
                    nc.tensor.matmul(
                        psB[:], xTs[64:128, k, jsl], wvs[64:128, k, :],
                        start=(k == 0), stop=(k == KT - 1),
                    )
                st = yp.tile([128, 512], F32, tag="y")
                nc.vector.tensor_copy(st[:], psA[:])
                nc.vector.tensor_add(
                    vs[:, j, 0:NHC * 65].rearrange("p (h c) -> p h c", c=65)[:, :, 0:64],
                    psB[:].rearrange("p (h d) -> p h d", h=NHC),
                    st[:].rearrange("p (h d) -> p h d", h=NHC),
                )

            def attention_jloop(h, qb, extras, prev_tail):
                m, p0 = h // 2, (h % 2) * 64
                av = av_ps.tile([128, QW], F32, tag="av")
                p_tiles = {}

                def emit_av(j):
                    for half in range(QW // 512):
                        nc.tensor.matmul(
                            av[:, half * 512:(half + 1) * 512],
                            vs[:, j, h * 65:h * 65 + 128],
                            p_tiles[j][:, half * 512:(half + 1) * 512],
                            start=(j == 0),
                            stop=(j == JT - 1),
                        )

                for j in range(JT):
                    # The two q-halves use complementary PE row-groups (via
                    # the swapped-half duplicates): K=64 matmuls on a single
                    # row-half never un-throttle the HAM clock gate (measured
                    # 434 ns/MM locked cold); complementary halves run warm
                    # AND concurrently (117 ns/MM).
                    sc = sc_ps.tile([128, QW], F32, tag="sc")
                    for half in range(QW // 512):
                        if half == 0:
                            kt, qt, r0 = kTs, qTs, p0
                        else:
                            kt, qt, r0 = kTs2, qTs2, 64 - p0
                        q0 = qb * QW + half * 512
                        nc.tensor.matmul(
                            sc[:, half * 512:(half + 1) * 512],
                            kt[r0:r0 + 64, m, j * 128:(j + 1) * 128],
                            qt[r0:r0 + 64, m, q0:q0 + 512],
                            start=True,
                            stop=True,
                        )
                    pt = pp.tile([128, QW], BF, tag="p")
                    nc.scalar.activation(pt[:], sc[:], AF.Exp, scale=SCALE)
                    p_tiles[j] = pt
                    if j < len(prev_tail):
                        prev_tail[j]()
                    for th in extras.get(j, []):
                        th()
                    if j > 0:
                        emit_av(j - 1)
                # The last AV matmul and the PSUM->SBUF eviction (which is
                # the only reader of the av PSUM slot) are deferred into the
                # next block's first two iterations so the block boundary
                # never stalls the exp stream.
                avs = asbp.tile([65, QW], F32, tag="asb")

                def tail0():
                    emit_av(JT - 1)
                    dmy = ex_ps.tile([128, 512], F32, tag="ex")
                    nc.tensor.matmul(
                        dmy[:], wqs[:, 0, 0:128], qTs[:, 0, 0:512],
                        start=True, stop=True,
                    )
                    nc.vector.tensor_copy(avs[:], av[0:65, :])

                return (avs, h, qb), [tail0]

            def normalize(blk):
                avs, h, qb = blk
                m, p0 = h // 2, (h % 2) * 64
                # DVE reciprocal is an 8-cycle/element iterative op on the
                # partition lanes; on a [1, QW] row it runs on ONE lane
                # (~6.5us).  Scatter the row across 64 lanes via DMA,
                # reciprocal there (~0.3us), and gather back.
                rc16 = rcp.tile([64, QW // 64], F32, tag="rc16")
                nc.gpsimd.dma_start(rc16[:], avs[64:65, :])
                nc.vector.reciprocal(rc16[:], rc16[:])
                rc = rcp.tile([1, QW], F32, tag="rcrow")
                nc.gpsimd.dma_start(rc[:], rc16[:])
                dst = attns[p0:p0 + 64, m, qb * QW:(qb + 1) * QW]
                tmp = None
                if p0 != 0:
                    tmp = tmpp.tile([64, QW], BF, tag="tmp")
                for half in range(QW // 512):
                    h0 = half * 512
                    bch = ex_ps.tile([64, 512], F32, tag="ex")
                    nc.tensor.matmul(
                        bch[:],
                        ones[0:1, :],
                        rc[0:1, h0:h0 + 512],
                        start=True,
                        stop=True,
                    )
                    out = dst[:, h0:h0 + 512] if tmp is None else tmp[:, h0:h0 + 512]
                    nc.vector.tensor_mul(out, avs[0:64, h0:h0 + 512], bch[:])
                if tmp is not None:
                    nc.gpsimd.dma_start(dst, tmp[:])

            def wo_chunk(qb, mo, n):
                q0 = qb * QW + n * 512
                mosl = slice(mo * 128, (mo + 1) * 128)
                psA = ex_ps.tile([128, 512], F32, tag="ex")
                psB = ex_ps.tile([128, 512], F32, tag="ex")
                for c in range(CT):
                    nc.tensor.matmul(
                        psA[:], wos[0:64, c, mosl], attns[0:64, c, q0:q0 + 512],
                        start=(c == 0), stop=(c == CT - 1),
                    )
                    nc.tensor.matmul(
                        psB[:], wos[64:128, c, mosl], attns[64:128, c, q0:q0 + 512],
                        start=(c == 0), stop=(c == CT - 1),
                    )
                yt = yp.tile([128, 512], F32, tag="y")
                nc.vector.tensor_scalar_add(yt[:], psA[:], bos[:, mo:mo + 1])
                nc.vector.tensor_add(yt[:], psB[:], yt[:])
                nc.sync.dma_start(yT_d[mosl, q0:q0 + 512], yt[:])

            # qk(m=0) and half of v upfront (they overlap the input DMAs);
            # later m's projections and the rest of v are interleaved into
            # the j-loops of earlier heads so the PE keeps feeding ACT
            # without a serial projection phase.
            for n in range(4):
                proj_qk(0, n, wqs, qTs, qTs2, bqs)
                proj_qk(0, n, wks, kTs, kTs2, bks)
            for j in range(JT):
                proj_v(j)

            def qk_thunks(m):
                qs = [
                    (lambda m=m, n=n: proj_qk(m, n, wqs, qTs, qTs2, bqs))
                    for n in range(4)
                ]
                ks = [
                    (lambda m=m, n=n: proj_qk(m, n, wks, kTs, kTs2, bks))
                    for n in range(4)
                ]
                return qs + ks

            extras_map = {}
            g1 = qk_thunks(1)
            extras_map[(0, 0)] = {4 * i: [g1[i]] for i in range(4)}
            extras_map[(0, 1)] = {4 * i: [g1[i + 4]] for i in range(4)}
            g2 = qk_thunks(2)
            extras_map[(0, 2)] = {4 * i: [g2[i]] for i in range(4)}
            extras_map[(0, 3)] = {4 * i: [g2[i + 4]] for i in range(4)}
            g3 = qk_thunks(3)
            extras_map[(0, 4)] = {4 * i: [g3[i]] for i in range(4)}
            extras_map[(0, 5)] = {4 * i: [g3[i + 4]] for i in range(4)}
            # Wo chunks for qb=0 ride inside qb=1's first two j-loops (they
            # only need qb0's attn rows, all normalized by then).
            # (qb0,h7)'s deferred normalize sits at extras[6] of (1,0), so
            # qb0's Wo chunks must come after it.
            wo0 = [(0, mo, n) for mo in range(KT) for n in range(QW // 512)]
            slots = (
                [(0, 7), (0, 9), (0, 11), (0, 13), (0, 15),
                 (1, 2), (1, 5), (1, 8), (1, 11), (1, 14),
                 (2, 12), (2, 14), (3, 12), (3, 14), (4, 13), (5, 13)]
            )
            for args, (hh, jj) in zip(wo0, slots):
                extras_map.setdefault((1, hh), {}).setdefault(jj, []).append(
                    lambda a=args: wo_chunk(*a)
                )

            pending, tail = None, []
            for qb in range(NQB):
                for h in range(NHC):
                    extras = {
                        j: list(ths)
                        for j, ths in extras_map.get((qb, h), {}).items()
                    }
                    if pending is not None:
                        # Normalize the previous block a few iterations into
                        # this block's j-loop: by then its SBUF eviction and
                        # reciprocal are long done, so the broadcast matmuls
                        # never stall the PE stream.
                        blk = pending
                        extras.setdefault(6, [])
                        extras[6] = [lambda b=blk: normalize(b)] + extras[6]
                    pending, tail = attention_jloop(h, qb, extras, tail)
            for th in tail:
                th()
            normalize(pending)
            for args in [(1, mo, n) for mo in range(KT) for n in range(QW // 512)]:
                wo_chunk(*args)

    nc.compile()
    _CACHE["nc"] = nc
    return nc


def _prep_core_inputs(x, Wq, bq, Wk, bk, Wv, bv, Wo, bo):
    bf16 = ml_dtypes.bfloat16
    bo_eff = (bo + bv @ Wo.T).astype(np.float32)
    in_maps = []
    for c in range(8):
        b, g = c // 2, c % 2
        sl = slice(g * HG, (g + 1) * HG)
        m = {
            "xT": np.ascontiguousarray(x[b].T).astype(bf16),
            "wq": np.ascontiguousarray(Wq[sl, :].T).astype(bf16),
            "wk": np.ascontiguousarray(Wk[sl, :].T).astype(bf16),
            "wv": np.ascontiguousarray(Wv[sl, :].T).astype(bf16),
            "wo": np.ascontiguousarray(Wo[:, sl].T).astype(bf16),
            "bq": np.ascontiguousarray(
                bq[sl].astype(np.float32).reshape(CT, 128).T
            ),
            "bk": np.ascontiguousarray(
                bk[sl].astype(np.float32).reshape(CT, 128).T
            ),
            "bo": np.ascontiguousarray(bo_eff.reshape(KT, 128).T)
            if g == 0
            else np.zeros((128, KT), np.float32),
        }
        in_maps.append(m)
    return in_maps


def kernel(x, Wq, bq, Wk, bk, Wv, bv, Wo, bo):
    global LAST_RESULT
    x = np.asarray(x, np.float32)
    nc = _build()
    in_maps = _prep_core_inputs(
        x,
        np.asarray(Wq, np.float32),
        np.asarray(bq, np.float32),
        np.asarray(Wk, np.float32),
        np.asarray(bk, np.float32),
        np.asarray(Wv, np.float32),
        np.asarray(bv, np.float32),
        np.asarray(Wo, np.float32),
        np.asarray(bo, np.float32),
    )
    res = run_bass_kernel_spmd(nc, in_maps, list(range(8)))
    LAST_RESULT = res
    out = np.empty((B, S, H), np.float32)
    for b in range(B):
        yT = res.results[2 * b]["yT"] + res.results[2 * b + 1]["yT"]
        out[b] = yT.T
    return out


# revision 46
# speedup vs baseline: 1.1982x; 1.1982x over previous
"""Multi-head attention (B=4, S=2048, H=1024, NH=16, D=64) on 8 trn2 cores.

Sharding: core c = (batch b = c//2, head-group g = c%2); each core owns 8
heads of one batch (tensor-parallel split of Wq/Wk/Wv columns + Wo rows,
data-parallel over batch). Each core computes the partial output
contribution yT_partial = Wo_g @ attn_g(b).T in transposed layout; the
host sums the two head-group partials per batch (the standard unshard for
a row-sharded Wo) and transposes back.

Device dataflow (all bf16 matmuls, fp32 PSUM accumulation):
  - host pre-transposes x and the weight slices so no on-device transpose
    of activations is ever needed:
      xT (H, S), wqT/wkT/wvT (H, HG), woT (HG, H)
  - qT = WqT.T-contracted projection -> (HG, S) transposed layout; same kT
  - v in natural layout (S, HG), stored as per-128-row j-tiles with a
    ones-column appended per head (the softmax denominator trick)
  - per (head, 1024-query block, 128-key tile j):
      scoresT_j = kT_h_j.T @ qT_h          [128 keys x 1024 queries] PSUM
      P_j = exp(scoresT_j / 8)             ScalarE, PSUM->SBUF bf16
      av += [v_j | 1 | pad].T @ P_j        [128 x 1024] PSUM (row 64 = denom)
    softmax skips the max-subtraction: q,k ~ N(0,1) so scores/8 ~ N(0,1),
    max |score| ~ 5.5 -> exp is far from overflow; result is identical to
    the stable softmax up to fp rounding.
  - normalize: denom row scattered to 64 lanes by DMA, DVE reciprocal
    (iterative 8 cyc/elem -- on one lane it would cost 6.5us), gathered
    back, broadcast across partitions via a K=1 f32 matmul against a ones
    row (the GPSIMD partition_broadcast op produces garbage on HW), then
    DVE multiply -> attn_sb transposed (HG, S) bf16.  Odd heads land on
    lanes 64..127 via an SBUF->SBUF DMA (DVE cannot cross partitions).
    Normalization of block i and its last AV matmul + PSUM eviction are
    deferred into block i+1's j-loop so block boundaries never stall the
    exp stream.
  - yT = woT-contracted projection + bias -> (H, S) f32 -> DRAM.

HAM clock-gate discipline (the big perf lever on trn2): K=64 or M<128
matmuls on a fixed row-half read as "low activity" and lock the PE at
half clock even when issued back-to-back.  Scores therefore run their two
query-halves on complementary row-groups (via swapped-half duplicates of
qT/kT), projections split their contraction into even/odd 64-row halves
in two PSUM banks, and the AV stationary operand is widened to the full
128 columns.

Biases: bq/bk are applied on-device (per-partition bias in transposed
layout). bv folds algebraically into the output bias: softmax rows sum to
1, so attn(v + 1*bv.T) = attn(v) + bv, hence y += bv @ Wo.T; the host
computes bo_eff = bo + bv @ Wo.T and the g==0 core adds it (g==1 gets
zeros so the host-side sum adds it exactly once).
"""

import sys

if "/opt/trn_rl_repo" not in sys.path:
    sys.path.insert(0, "/opt/trn_rl_repo")

import ml_dtypes
import numpy as np

import concourse.bass as bass
import concourse.tile as tile
from concourse import bacc, mybir
from concourse.bass_utils import run_bass_kernel_spmd

BF = mybir.dt.bfloat16
F32 = mybir.dt.float32
AF = mybir.ActivationFunctionType

B, S, H = 4, 2048, 1024
NH, D = 16, 64
HG = H // 2          # head-group width (8 heads)
NHC = 8              # heads per core
KT = H // 128        # k-tiles over H contraction
CT = HG // 128       # tiles over head-group dims
JT = S // 128        # key tiles
QW = 1024            # query-block width
NQB = S // QW
SCALE = 0.125        # 1/sqrt(D)

_CACHE = {}
LAST_RESULT = None


def _build():
    if "nc" in _CACHE:
        return _CACHE["nc"]

    nc = bacc.Bacc("TRN2", target_bir_lowering=False, debug=False, num_devices=8)

    xT_d = nc.dram_tensor("xT", [H, S], BF, kind="ExternalInput")
    wq_d = nc.dram_tensor("wq", [H, HG], BF, kind="ExternalInput")
    wk_d = nc.dram_tensor("wk", [H, HG], BF, kind="ExternalInput")
    wv_d = nc.dram_tensor("wv", [H, HG], BF, kind="ExternalInput")
    wo_d = nc.dram_tensor("wo", [HG, H], BF, kind="ExternalInput")
    bq_d = nc.dram_tensor("bq", [128, CT], F32, kind="ExternalInput")
    bk_d = nc.dram_tensor("bk", [128, CT], F32, kind="ExternalInput")
    bo_d = nc.dram_tensor("bo", [128, KT], F32, kind="ExternalInput")
    yT_d = nc.dram_tensor("yT", [H, S], F32, kind="ExternalOutput")

    with tile.TileContext(nc) as tc:
        with (
            tc.tile_pool(name="const", bufs=1) as constp,
            tc.tile_pool(name="ex_ps", bufs=2, space=bass.MemorySpace.PSUM) as ex_ps,
            tc.tile_pool(name="sc_ps", bufs=2, space=bass.MemorySpace.PSUM) as sc_ps,
            tc.tile_pool(name="av_ps", bufs=1, space=bass.MemorySpace.PSUM) as av_ps,
            tc.tile_pool(name="pp", bufs=4) as pp,
            tc.tile_pool(name="rcp", bufs=1) as rcp,
            tc.tile_pool(name="asb", bufs=2) as asbp,
            tc.tile_pool(name="tmpp", bufs=2) as tmpp,
            tc.tile_pool(name="yp", bufs=2) as yp,
        ):
            xTs = constp.tile([128, KT, S], BF)
            wqs = constp.tile([128, KT, HG], BF)
            wks = constp.tile([128, KT, HG], BF)
            wvs = constp.tile([128, KT, HG], BF)
            wos = constp.tile([128, CT, H], BF)
            bqs = constp.tile([128, CT], F32)
            bks = constp.tile([128, CT], F32)
            bos = constp.tile([128, KT], F32)
            ones = constp.tile([65, 64], F32)
            warm = constp.tile([1, 8], F32)
            qTs = constp.tile([128, CT, S], BF)
            kTs = constp.tile([128, CT, S], BF)
            # Swapped-half duplicates (partitions 0:64 <-> 64:128): the two
            # query-halves of each scores tile run on complementary PE
            # row-groups, overlapping their LDWEIGHTS and keeping the HAM
            # clock-gate warm (K=64 on a fixed half locks the PE cold).
            qTs2 = constp.tile([128, CT, S], BF)
            kTs2 = constp.tile([128, CT, S], BF)
            # Per j-tile: 8 head-blocks of [64 v-dims | ones]; 64 columns of
            # padding at the end so every head's AV lhsT can be widened to
            # 128 columns (head h reads cols h*65 .. h*65+128 — the extra 63
            # columns produce discarded rows but keep the PE array at full
            # column activity, which the HAM clock-gate needs to stay warm).
            vs = constp.tile([128, JT, NHC * 65 + 64], BF)
            attns = constp.tile([128, CT, S], BF)

            # Preload the ACT exp table while input DMAs run.
            nc.vector.memset(warm[:], 0.0)
            nc.scalar.activation(warm[:], warm[:], AF.Exp)

            nc.sync.dma_start(bqs[:], bq_d[:])
            nc.sync.dma_start(bks[:], bk_d[:])
            nc.sync.dma_start(bos[:], bo_d[:])
            for k in range(KT):
                nc.sync.dma_start(wqs[:, k, :], wq_d[k * 128:(k + 1) * 128, :])
                nc.sync.dma_start(wks[:, k, :], wk_d[k * 128:(k + 1) * 128, :])
                nc.sync.dma_start(
                    xTs[:, k, 0:S // 2], xT_d[k * 128:(k + 1) * 128, 0:S // 2]
                )
            for k in range(KT):
                nc.sync.dma_start(
                    xTs[:, k, S // 2:], xT_d[k * 128:(k + 1) * 128, S // 2:]
                )
            for k in range(KT):
                nc.sync.dma_start(wvs[:, k, :], wv_d[k * 128:(k + 1) * 128, :])
            for c in range(CT):
                nc.sync.dma_start(wos[:, c, :], wo_d[c * 128:(c + 1) * 128, :])

            nc.vector.memset(
                vs[:, :, 0:NHC * 65].rearrange("p j (h c) -> p j h c", c=65)[:, :, :, 64],
                1.0,
            )
            nc.vector.memset(vs[:, :, NHC * 65:], 1.0)
            nc.vector.memset(ones[:], 1.0)

            # All projections split the 128-row contraction tiles into
            # even/odd 64-row halves accumulating in two 1-bank PSUM tiles:
            # consecutive matmuls alternate PE row-groups, so each
            # LDWEIGHTS overlaps the in-flight matmul instead of
            # serializing (~107 ns/MM saved).  A DVE pass combines the
            # halves (via an f32 staging tile to avoid double rounding).
            def proj_qk(m, n, ws, dst, dst2, bias):
                msl = slice(m * 128, (m + 1) * 128)
                nsl = slice(n * 512, (n + 1) * 512)
                psA = ex_ps.tile([128, 512], F32, tag="ex")
                psB = ex_ps.tile([128, 512], F32, tag="ex")
                for k in range(KT):
                    nc.tensor.matmul(
                        psA[:], ws[0:64, k, msl], xTs[0:64, k, nsl],
                        start=(k == 0), stop=(k == KT - 1),
                    )
                    nc.tensor.matmul(
                        psB[:], ws[64:128, k, msl], xTs[64:128, k, nsl],
                        start=(k == 0), stop=(k == KT - 1),
                    )
                st = yp.tile([128, 512], F32, tag="y")
                nc.vector.tensor_scalar_add(st[:], psA[:], bias[:, m:m + 1])
                nc.vector.tensor_add(dst[:, m, nsl], psB[:], st[:])
                nc.gpsimd.dma_start(dst2[64:128, m, nsl], dst[0:64, m, nsl])
                nc.gpsimd.dma_start(dst2[0:64, m, nsl], dst[64:128, m, nsl])

            def proj_v(j):
                jsl = slice(j * 128, (j + 1) * 128)
                psA = ex_ps.tile([128, 512], F32, tag="ex")
                psB = ex_ps.tile([128, 512], F32, tag="ex")
                for k in range(KT):
                    nc.tensor.matmul(
                        psA[:], xTs[0:64, k, jsl], wvs[0:64, k, :],
                        start=(k == 0), stop=(k == KT - 1),
                    )
                    nc.tensor.matmul(
                        psB[:], xTs[64:128, k, jsl], wvs[64:128, k, :],
                        start=(k == 0), stop=(k == KT - 1),
                    )
                st = yp.tile([128, 512], F32, tag="y")
                nc.vector.tensor_copy(st[:], psA[:])
                nc.vector.tensor_add(
                    vs[:, j, 0:NHC * 65].rearrange("p (h c) -> p h c", c=65)[:, :, 0:64],
                    psB[:].rearrange("p (h d) -> p h d", h=NHC),
                    st[:].rearrange("p (h d) -> p h d", h=NHC),
                )

            def attention_jloop(h, qb, extras, prev_tail):
                m, p0 = h // 2, (h % 2) * 64
                av = av_ps.tile([128, QW], F32, tag="av")
                p_tiles = {}

                def emit_av(j):
                    for half in range(QW // 512):
                        nc.tensor.matmul(
                            av[:, half * 512:(half + 1) * 512],
                            vs[:, j, h * 65:h * 65 + 128],
                            p_tiles[j][:, half * 512:(half + 1) * 512],
                            start=(j == 0),
                            stop=(j == JT - 1),
                        )

                for j in range(JT):
                    # The two q-halves use complementary PE row-groups (via
                    # the swapped-half duplicates): K=64 matmuls on a single
                    # row-half never un-throttle the HAM clock gate (measured
                    # 434 ns/MM locked cold); complementary halves run warm
                    # AND concurrently (117 ns/MM).
                    sc = sc_ps.tile([128, QW], F32, tag="sc")
                    for half in range(QW // 512):
                        if half == 0:
                            kt, qt, r0 = kTs, qTs, p0
                        else:
                            kt, qt, r0 = kTs2, qTs2, 64 - p0
                        q0 = qb * QW + half * 512
                        nc.tensor.matmul(
                            sc[:, half * 512:(half + 1) * 512],
                            kt[r0:r0 + 64, m, j * 128:(j + 1) * 128],
                            qt[r0:r0 + 64, m, q0:q0 + 512],
                            start=True,
                            stop=True,
                        )
                    pt = pp.tile([128, QW], BF, tag="p")
                    nc.scalar.activation(pt[:], sc[:], AF.Exp, scale=SCALE)
                    p_tiles[j] = pt
                    if j < len(prev_tail):
                        prev_tail[j]()
                    for th in extras.get(j, []):
                        th()
                    if j > 0:
                        emit_av(j - 1)
                # The last AV matmul and the PSUM->SBUF eviction (which is
                # the only reader of the av PSUM slot) are deferred into the
                # next block's first two iterations so the block boundary
                # never stalls the exp stream.
                avs = asbp.tile([65, QW], F32, tag="asb")

                def tail0():
                    emit_av(JT - 1)
                    dmy = ex_ps.tile([128, 512], F32, tag="ex")
                    nc.tensor.matmul(
                        dmy[:], wqs[:, 0, 0:128], qTs[:, 0, 0:512],
                        start=True, stop=True,
                    )
                    nc.vector.tensor_copy(avs[:], av[0:65, :])

                return (avs, h, qb), [tail0]

            def normalize(blk):
                avs, h, qb = blk
                m, p0 = h // 2, (h % 2) * 64
                # DVE reciprocal is an 8-cycle/element iterative op on the
                # partition lanes; on a [1, QW] row it runs on ONE lane
                # (~6.5us).  Scatter the row across 64 lanes via DMA,
                # reciprocal there (~0.3us), and gather back.
                rc16 = rcp.tile([64, QW // 64], F32, tag="rc16")
                nc.gpsimd.dma_start(rc16[:], avs[64:65, :])
                nc.vector.reciprocal(rc16[:], rc16[:])
                rc = rcp.tile([1, QW], F32, tag="rcrow")
                nc.gpsimd.dma_start(rc[:], rc16[:])
                dst = attns[p0:p0 + 64, m, qb * QW:(qb + 1) * QW]
                tmp = None
                if p0 != 0:
                    tmp = tmpp.tile([64, QW], BF, tag="tmp")
                for half in range(QW // 512):
                    h0 = half * 512
                    bch = ex_ps.tile([64, 512], F32, tag="ex")
                    nc.tensor.matmul(
                        bch[:],
                        ones[0:1, :],
                        rc[0:1, h0:h0 + 512],
                        start=True,
                        stop=True,
                    )
                    out = dst[:, h0:h0 + 512] if tmp is None else tmp[:, h0:h0 + 512]
                    nc.vector.tensor_mul(out, avs[0:64, h0:h0 + 512], bch[:])
                # K=1 broadcast matmuls read as near-idle to the HAM gate;
                # chase them with one full-K matmul to hold the clock.
                dmy = ex_ps.tile([128, 512], F32, tag="ex")
                nc.tensor.matmul(
                    dmy[:], wqs[:, 0, 0:128], qTs[:, 0, 0:512],
                    start=True, stop=True,
                )
                if tmp is not None:
                    nc.gpsimd.dma_start(dst, tmp[:])

            def wo_chunk(qb, mo, n):
                q0 = qb * QW + n * 512
                mosl = slice(mo * 128, (mo + 1) * 128)
                psA = ex_ps.tile([128, 512], F32, tag="ex")
                psB = ex_ps.tile([128, 512], F32, tag="ex")
                for c in range(CT):
                    nc.tensor.matmul(
                        psA[:], wos[0:64, c, mosl], attns[0:64, c, q0:q0 + 512],
                        start=(c == 0), stop=(c == CT - 1),
                    )
                    nc.tensor.matmul(
                        psB[:], wos[64:128, c, mosl], attns[64:128, c, q0:q0 + 512],
                        start=(c == 0), stop=(c == CT - 1),
                    )
                yt = yp.tile([128, 512], F32, tag="y")
                nc.vector.tensor_scalar_add(yt[:], psA[:], bos[:, mo:mo + 1])
                nc.vector.tensor_add(yt[:], psB[:], yt[:])
                nc.sync.dma_start(yT_d[mosl, q0:q0 + 512], yt[:])

            # qk(m=0) and half of v upfront (they overlap the input DMAs);
            # later m's projections and the rest of v are interleaved into
            # the j-loops of earlier heads so the PE keeps feeding ACT
            # without a serial projection phase.
            for n in range(4):
                proj_qk(0, n, wqs, qTs, qTs2, bqs)
                proj_qk(0, n, wks, kTs, kTs2, bks)
            for j in range(JT):
                proj_v(j)

            def qk_thunks(m):
                qs = [
                    (lambda m=m, n=n: proj_qk(m, n, wqs, qTs, qTs2, bqs))
                    for n in range(4)
                ]
                ks = [
                    (lambda m=m, n=n: proj_qk(m, n, wks, kTs, kTs2, bks))
                    for n in range(4)
                ]
                return qs + ks

            extras_map = {}
            g1 = qk_thunks(1)
            extras_map[(0, 0)] = {4 * i: [g1[i]] for i in range(4)}
            extras_map[(0, 1)] = {4 * i: [g1[i + 4]] for i in range(4)}
            g2 = qk_thunks(2)
            extras_map[(0, 2)] = {4 * i: [g2[i]] for i in range(4)}
            extras_map[(0, 3)] = {4 * i: [g2[i + 4]] for i in range(4)}
            g3 = qk_thunks(3)
            extras_map[(0, 4)] = {4 * i: [g3[i]] for i in range(4)}
            extras_map[(0, 5)] = {4 * i: [g3[i + 4]] for i in range(4)}
            # Wo chunks for qb=0 ride inside qb=1's first two j-loops (they
            # only need qb0's attn rows, all normalized by then).
            # (qb0,h7)'s deferred normalize sits at extras[6] of (1,0), so
            # qb0's Wo chunks must come after it.
            wo0 = [(0, mo, n) for mo in range(KT) for n in range(QW // 512)]
            slots = (
                [(0, 7), (0, 9), (0, 11), (0, 13), (0, 15),
                 (1, 2), (1, 5), (1, 8), (1, 11), (1, 14),
                 (2, 12), (2, 14), (3, 12), (3, 14), (4, 13), (5, 13)]
            )
            for args, (hh, jj) in zip(wo0, slots):
                extras_map.setdefault((1, hh), {}).setdefault(jj, []).append(
                    lambda a=args: wo_chunk(*a)
                )

            pending, tail = None, []
            for qb in range(NQB):
                for h in range(NHC):
                    extras = {
                        j: list(ths)
                        for j, ths in extras_map.get((qb, h), {}).items()
                    }
                    if pending is not None:
                        # Normalize the previous block a few iterations into
                        # this block's j-loop: by then its SBUF eviction and
                        # reciprocal are long done, so the broadcast matmuls
                        # never stall the PE stream.
                        blk = pending
                        extras.setdefault(6, [])
                        extras[6] = [lambda b=blk: normalize(b)] + extras[6]
                    pending, tail = attention_jloop(h, qb, extras, tail)
            for th in tail:
                th()
            normalize(pending)
            for args in [(1, mo, n) for mo in range(KT) for n in range(QW // 512)]:
                wo_chunk(*args)

    nc.compile()
    _CACHE["nc"] = nc
    return nc


def _prep_core_inputs(x, Wq, bq, Wk, bk, Wv, bv, Wo, bo):
    bf16 = ml_dtypes.bfloat16
    bo_eff = (bo + bv @ Wo.T).astype(np.float32)
    in_maps = []
    for c in range(8):
        b, g = c // 2, c % 2
        sl = slice(g * HG, (g + 1) * HG)
        m = {
            "xT": np.ascontiguousarray(x[b].T).astype(bf16),
            "wq": np.ascontiguousarray(Wq[sl, :].T).astype(bf16),
            "wk": np.ascontiguousarray(Wk[sl, :].T).astype(bf16),
            "wv": np.ascontiguousarray(Wv[sl, :].T).astype(bf16),
            "wo": np.ascontiguousarray(Wo[:, sl].T).astype(bf16),
            "bq": np.ascontiguousarray(
                bq[sl].astype(np.float32).reshape(CT, 128).T
            ),
            "bk": np.ascontiguousarray(
                bk[sl].astype(np.float32).reshape(CT, 128).T
            ),
            "bo": np.ascontiguousarray(bo_eff.reshape(KT, 128).T)
            if g == 0
            else np.zeros((128, KT), np.float32),
        }
        in_maps.append(m)
    return in_maps


def kernel(x, Wq, bq, Wk, bk, Wv, bv, Wo, bo):
    global LAST_RESULT
    x = np.asarray(x, np.float32)
    nc = _build()
    in_maps = _prep_core_inputs(
        x,
        np.asarray(Wq, np.float32),
        np.asarray(bq, np.float32),
        np.asarray(Wk, np.float32),
        np.asarray(bk, np.float32),
        np.asarray(Wv, np.float32),
        np.asarray(bv, np.float32),
        np.asarray(Wo, np.float32),
        np.asarray(bo, np.float32),
    )
    res = run_bass_kernel_spmd(nc, in_maps, list(range(8)))
    LAST_RESULT = res
    out = np.empty((B, S, H), np.float32)
    for b in range(B):
        yT = res.results[2 * b]["yT"] + res.results[2 * b + 1]["yT"]
        out[b] = yT.T
    return out


# revision 47
# speedup vs baseline: 1.2002x; 1.0017x over previous
"""Multi-head attention (B=4, S=2048, H=1024, NH=16, D=64) on 8 trn2 cores.

Sharding: core c = (batch b = c//2, head-group g = c%2); each core owns 8
heads of one batch (tensor-parallel split of Wq/Wk/Wv columns + Wo rows,
data-parallel over batch). Each core computes the partial output
contribution yT_partial = Wo_g @ attn_g(b).T in transposed layout; the
host sums the two head-group partials per batch (the standard unshard for
a row-sharded Wo) and transposes back.

Device dataflow (all bf16 matmuls, fp32 PSUM accumulation):
  - host pre-transposes x and the weight slices so no on-device transpose
    of activations is ever needed:
      xT (H, S), wqT/wkT/wvT (H, HG), woT (HG, H)
  - qT = WqT.T-contracted projection -> (HG, S) transposed layout; same kT
  - v in natural layout (S, HG), stored as per-128-row j-tiles with a
    ones-column appended per head (the softmax denominator trick)
  - per (head, 1024-query block, 128-key tile j):
      scoresT_j = kT_h_j.T @ qT_h          [128 keys x 1024 queries] PSUM
      P_j = exp(scoresT_j / 8)             ScalarE, PSUM->SBUF bf16
      av += [v_j | 1 | pad].T @ P_j        [128 x 1024] PSUM (row 64 = denom)
    softmax skips the max-subtraction: q,k ~ N(0,1) so scores/8 ~ N(0,1),
    max |score| ~ 5.5 -> exp is far from overflow; result is identical to
    the stable softmax up to fp rounding.
  - normalize: denom row scattered to 64 lanes by DMA, DVE reciprocal
    (iterative 8 cyc/elem -- on one lane it would cost 6.5us), gathered
    back, broadcast across partitions via a K=1 f32 matmul against a ones
    row (the GPSIMD partition_broadcast op produces garbage on HW), then
    DVE multiply -> attn_sb transposed (HG, S) bf16.  Odd heads land on
    lanes 64..127 via an SBUF->SBUF DMA (DVE cannot cross partitions).
    Normalization of block i and its last AV matmul + PSUM eviction are
    deferred into block i+1's j-loop so block boundaries never stall the
    exp stream.
  - yT = woT-contracted projection + bias -> (H, S) f32 -> DRAM.

HAM clock-gate discipline (the big perf lever on trn2): K=64 or M<128
matmuls on a fixed row-half read as "low activity" and lock the PE at
half clock even when issued back-to-back.  Scores therefore run their two
query-halves on complementary row-groups (via swapped-half duplicates of
qT/kT), projections split their contraction into even/odd 64-row halves
in two PSUM banks, and the AV stationary operand is widened to the full
128 columns.

Biases: bq/bk are applied on-device (per-partition bias in transposed
layout). bv folds algebraically into the output bias: softmax rows sum to
1, so attn(v + 1*bv.T) = attn(v) + bv, hence y += bv @ Wo.T; the host
computes bo_eff = bo + bv @ Wo.T and the g==0 core adds it (g==1 gets
zeros so the host-side sum adds it exactly once).
"""

import sys

if "/opt/trn_rl_repo" not in sys.path:
    sys.path.insert(0, "/opt/trn_rl_repo")

import ml_dtypes
import numpy as np

import concourse.bass as bass
import concourse.tile as tile
from concourse import bacc, mybir
from concourse.bass_utils import run_bass_kernel_spmd

BF = mybir.dt.bfloat16
F32 = mybir.dt.float32
AF = mybir.ActivationFunctionType

B, S, H = 4, 2048, 1024
NH, D = 16, 64
HG = H // 2          # head-group width (8 heads)
NHC = 8              # heads per core
KT = H // 128        # k-tiles over H contraction
CT = HG // 128       # tiles over head-group dims
JT = S // 128        # key tiles
QW = 1024            # query-block width
NQB = S // QW
SCALE = 0.125        # 1/sqrt(D)

_CACHE = {}
LAST_RESULT = None


def _build():
    if "nc" in _CACHE:
        return _CACHE["nc"]

    nc = bacc.Bacc("TRN2", target_bir_lowering=False, debug=False, num_devices=8)

    xT_d = nc.dram_tensor("xT", [H, S], BF, kind="ExternalInput")
    wq_d = nc.dram_tensor("wq", [H, HG], BF, kind="ExternalInput")
    wk_d = nc.dram_tensor("wk", [H, HG], BF, kind="ExternalInput")
    wv_d = nc.dram_tensor("wv", [H, HG], BF, kind="ExternalInput")
    wo_d = nc.dram_tensor("wo", [HG, H], BF, kind="ExternalInput")
    bq_d = nc.dram_tensor("bq", [128, CT], F32, kind="ExternalInput")
    bk_d = nc.dram_tensor("bk", [128, CT], F32, kind="ExternalInput")
    bo_d = nc.dram_tensor("bo", [128, KT], F32, kind="ExternalInput")
    yT_d = nc.dram_tensor("yT", [H, S], F32, kind="ExternalOutput")

    with tile.TileContext(nc) as tc:
        with (
            tc.tile_pool(name="const", bufs=1) as constp,
            tc.tile_pool(name="ex_ps", bufs=2, space=bass.MemorySpace.PSUM) as ex_ps,
            tc.tile_pool(name="sc_ps", bufs=2, space=bass.MemorySpace.PSUM) as sc_ps,
            tc.tile_pool(name="av_ps", bufs=1, space=bass.MemorySpace.PSUM) as av_ps,
            tc.tile_pool(name="pp", bufs=4) as pp,
            tc.tile_pool(name="rcp", bufs=1) as rcp,
            tc.tile_pool(name="asb", bufs=2) as asbp,
            tc.tile_pool(name="tmpp", bufs=2) as tmpp,
            tc.tile_pool(name="yp", bufs=2) as yp,
        ):
            xTs = constp.tile([128, KT, S], BF)
            wqs = constp.tile([128, KT, HG], BF)
            wks = constp.tile([128, KT, HG], BF)
            wvs = constp.tile([128, KT, HG], BF)
            wos = constp.tile([128, CT, H], BF)
            bqs = constp.tile([128, CT], F32)
            bks = constp.tile([128, CT], F32)
            bos = constp.tile([128, KT], F32)
            ones = constp.tile([65, 64], F32)
            warm = constp.tile([1, 8], F32)
            qTs = constp.tile([128, CT, S], BF)
            kTs = constp.tile([128, CT, S], BF)
            # Swapped-half duplicates (partitions 0:64 <-> 64:128): the two
            # query-halves of each scores tile run on complementary PE
            # row-groups, overlapping their LDWEIGHTS and keeping the HAM
            # clock-gate warm (K=64 on a fixed half locks the PE cold).
            qTs2 = constp.tile([128, CT, S], BF)
            kTs2 = constp.tile([128, CT, S], BF)
            # Per j-tile: 8 head-blocks of [64 v-dims | ones]; 64 columns of
            # padding at the end so every head's AV lhsT can be widened to
            # 128 columns (head h reads cols h*65 .. h*65+128 — the extra 63
            # columns produce discarded rows but keep the PE array at full
            # column activity, which the HAM clock-gate needs to stay warm).
            vs = constp.tile([128, JT, NHC * 65 + 64], BF)
            attns = constp.tile([128, CT, S], BF)

            # Preload the ACT exp table while input DMAs run.
            nc.vector.memset(warm[:], 0.0)
            nc.scalar.activation(warm[:], warm[:], AF.Exp)

            nc.sync.dma_start(bqs[:], bq_d[:])
            nc.sync.dma_start(bks[:], bk_d[:])
            nc.sync.dma_start(bos[:], bo_d[:])
            for k in range(KT):
                nc.sync.dma_start(wqs[:, k, :], wq_d[k * 128:(k + 1) * 128, :])
                nc.sync.dma_start(wks[:, k, :], wk_d[k * 128:(k + 1) * 128, :])
                nc.sync.dma_start(
                    xTs[:, k, 0:S // 2], xT_d[k * 128:(k + 1) * 128, 0:S // 2]
                )
            for k in range(KT):
                nc.sync.dma_start(
                    xTs[:, k, S // 2:], xT_d[k * 128:(k + 1) * 128, S // 2:]
                )
            for k in range(KT):
                nc.sync.dma_start(wvs[:, k, :], wv_d[k * 128:(k + 1) * 128, :])
            for c in range(CT):
                nc.sync.dma_start(wos[:, c, :], wo_d[c * 128:(c + 1) * 128, :])

            nc.vector.memset(
                vs[:, :, 0:NHC * 65].rearrange("p j (h c) -> p j h c", c=65)[:, :, :, 64],
                1.0,
            )
            nc.vector.memset(vs[:, :, NHC * 65:], 1.0)
            nc.vector.memset(ones[:], 1.0)

            # All projections split the 128-row contraction tiles into
            # even/odd 64-row halves accumulating in two 1-bank PSUM tiles:
            # consecutive matmuls alternate PE row-groups, so each
            # LDWEIGHTS overlaps the in-flight matmul instead of
            # serializing (~107 ns/MM saved).  A DVE pass combines the
            # halves (via an f32 staging tile to avoid double rounding).
            def proj_qk(m, n, ws, dst, dst2, bias):
                msl = slice(m * 128, (m + 1) * 128)
                nsl = slice(n * 512, (n + 1) * 512)
                psA = ex_ps.tile([128, 512], F32, tag="ex")
                psB = ex_ps.tile([128, 512], F32, tag="ex")
                for k in range(KT):
                    nc.tensor.matmul(
                        psA[:], ws[0:64, k, msl], xTs[0:64, k, nsl],
                        start=(k == 0), stop=(k == KT - 1),
                    )
                    nc.tensor.matmul(
                        psB[:], ws[64:128, k, msl], xTs[64:128, k, nsl],
                        start=(k == 0), stop=(k == KT - 1),
                    )
                st = yp.tile([128, 512], F32, tag="y")
                nc.vector.tensor_scalar_add(st[:], psA[:], bias[:, m:m + 1])
                nc.vector.tensor_add(dst[:, m, nsl], psB[:], st[:])
                nc.gpsimd.dma_start(dst2[64:128, m, nsl], dst[0:64, m, nsl])
                nc.gpsimd.dma_start(dst2[0:64, m, nsl], dst[64:128, m, nsl])

            def proj_v(j):
                jsl = slice(j * 128, (j + 1) * 128)
                psA = ex_ps.tile([128, 512], F32, tag="ex")
                psB = ex_ps.tile([128, 512], F32, tag="ex")
                for k in range(KT):
                    nc.tensor.matmul(
                        psA[:], xTs[0:64, k, jsl], wvs[0:64, k, :],
                        start=(k == 0), stop=(k == KT - 1),
                    )
                    nc.tensor.matmul(
                        psB[:], xTs[64:128, k, jsl], wvs[64:128, k, :],
                        start=(k == 0), stop=(k == KT - 1),
                    )
                st = yp.tile([128, 512], F32, tag="y")
                nc.vector.tensor_copy(st[:], psA[:])
                nc.vector.tensor_add(
                    vs[:, j, 0:NHC * 65].rearrange("p (h c) -> p h c", c=65)[:, :, 0:64],
                    psB[:].rearrange("p (h d) -> p h d", h=NHC),
                    st[:].rearrange("p (h d) -> p h d", h=NHC),
                )

            def attention_jloop(h, qb, extras, prev_tail):
                m, p0 = h // 2, (h % 2) * 64
                av = av_ps.tile([128, QW], F32, tag="av")
                p_tiles = {}

                def emit_av(j):
                    for half in range(QW // 512):
                        nc.tensor.matmul(
                            av[:, half * 512:(half + 1) * 512],
                            vs[:, j, h * 65:h * 65 + 128],
                            p_tiles[j][:, half * 512:(half + 1) * 512],
                            start=(j == 0),
                            stop=(j == JT - 1),
                        )

                for j in range(JT):
                    # The two q-halves use complementary PE row-groups (via
                    # the swapped-half duplicates): K=64 matmuls on a single
                    # row-half never un-throttle the HAM clock gate (measured
                    # 434 ns/MM locked cold); complementary halves run warm
                    # AND concurrently (117 ns/MM).
                    sc = sc_ps.tile([128, QW], F32, tag="sc")
                    for half in range(QW // 512):
                        if half == 0:
                            kt, qt, r0 = kTs, qTs, p0
                        else:
                            kt, qt, r0 = kTs2, qTs2, 64 - p0
                        q0 = qb * QW + half * 512
                        nc.tensor.matmul(
                            sc[:, half * 512:(half + 1) * 512],
                            kt[r0:r0 + 64, m, j * 128:(j + 1) * 128],
                            qt[r0:r0 + 64, m, q0:q0 + 512],
                            start=True,
                            stop=True,
                        )
                    pt = pp.tile([128, QW], BF, tag="p")
                    nc.scalar.activation(pt[:], sc[:], AF.Exp, scale=SCALE)
                    p_tiles[j] = pt
                    if j < len(prev_tail):
                        prev_tail[j]()
                    for th in extras.get(j, []):
                        th()
                    if j > 0:
                        emit_av(j - 1)
                # The last AV matmul and the PSUM->SBUF eviction (which is
                # the only reader of the av PSUM slot) are deferred into the
                # next block's first two iterations so the block boundary
                # never stalls the exp stream.
                avs = asbp.tile([65, QW], F32, tag="asb")

                def tail0():
                    emit_av(JT - 1)
                    dmy = ex_ps.tile([128, 512], F32, tag="ex")
                    nc.tensor.matmul(
                        dmy[:], wqs[:, 0, 0:128], qTs[:, 0, 0:512],
                        start=True, stop=True,
                    )
                    nc.vector.tensor_copy(avs[:], av[0:65, :])

                return (avs, h, qb), [tail0]

            def normalize(blk):
                avs, h, qb = blk
                m, p0 = h // 2, (h % 2) * 64
                # DVE reciprocal is an 8-cycle/element iterative op on the
                # partition lanes; on a [1, QW] row it runs on ONE lane
                # (~6.5us).  Scatter the row across 64 lanes via DMA,
                # reciprocal there (~0.3us), and gather back.
                rc16 = rcp.tile([64, QW // 64], F32, tag="rc16")
                nc.gpsimd.dma_start(rc16[:], avs[64:65, :])
                nc.vector.reciprocal(rc16[:], rc16[:])
                rc = rcp.tile([1, QW], F32, tag="rcrow")
                nc.gpsimd.dma_start(rc[:], rc16[:])
                dst = attns[p0:p0 + 64, m, qb * QW:(qb + 1) * QW]
                tmp = None
                if p0 != 0:
                    tmp = tmpp.tile([64, QW], BF, tag="tmp")
                for half in range(QW // 512):
                    h0 = half * 512
                    bch = ex_ps.tile([64, 512], F32, tag="ex")
                    nc.tensor.matmul(
                        bch[:],
                        ones[0:1, :],
                        rc[0:1, h0:h0 + 512],
                        start=True,
                        stop=True,
                    )
                    out = dst[:, h0:h0 + 512] if tmp is None else tmp[:, h0:h0 + 512]
                    nc.vector.tensor_mul(out, avs[0:64, h0:h0 + 512], bch[:])
                # K=1 broadcast matmuls read as near-idle to the HAM gate;
                # chase them with one full-K matmul to hold the clock.
                dmy = ex_ps.tile([128, 512], F32, tag="ex")
                nc.tensor.matmul(
                    dmy[:], wqs[:, 0, 0:128], qTs[:, 0, 0:512],
                    start=True, stop=True,
                )
                if tmp is not None:
                    nc.gpsimd.dma_start(dst, tmp[:])

            def wo_chunk(qb, mo, n):
                q0 = qb * QW + n * 512
                mosl = slice(mo * 128, (mo + 1) * 128)
                psA = ex_ps.tile([128, 512], F32, tag="ex")
                psB = ex_ps.tile([128, 512], F32, tag="ex")
                for c in range(CT):
                    nc.tensor.matmul(
                        psA[:], wos[0:64, c, mosl], attns[0:64, c, q0:q0 + 512],
                        start=(c == 0), stop=(c == CT - 1),
                    )
                    nc.tensor.matmul(
                        psB[:], wos[64:128, c, mosl], attns[64:128, c, q0:q0 + 512],
                        start=(c == 0), stop=(c == CT - 1),
                    )
                yt = yp.tile([128, 512], F32, tag="y")
                nc.vector.tensor_scalar_add(yt[:], psA[:], bos[:, mo:mo + 1])
                nc.vector.tensor_add(yt[:], psB[:], yt[:])
                nc.sync.dma_start(yT_d[mosl, q0:q0 + 512], yt[:])

            # qk(m=0) and half of v upfront (they overlap the input DMAs);
            # later m's projections and the rest of v are interleaved into
            # the j-loops of earlier heads so the PE keeps feeding ACT
            # without a serial projection phase.
            for n in range(4):
                proj_qk(0, n, wqs, qTs, qTs2, bqs)
                proj_qk(0, n, wks, kTs, kTs2, bks)
            for j in range(JT):
                proj_v(j)

            def qk_thunks(m):
                qs = [
                    (lambda m=m, n=n: proj_qk(m, n, wqs, qTs, qTs2, bqs))
                    for n in range(4)
                ]
                ks = [
                    (lambda m=m, n=n: proj_qk(m, n, wks, kTs, kTs2, bks))
                    for n in range(4)
                ]
                return qs + ks

            def warm_mm():
                dmy = ex_ps.tile([128, 512], F32, tag="ex")
                nc.tensor.matmul(
                    dmy[:], wqs[:, 0, 0:128], qTs[:, 0, 0:512],
                    start=True, stop=True,
                )

            extras_map = {}
            # Sparse blocks (no projection filler) get two warm-keepers each
            # to hold the HAM clock through their emptiest stretches.
            for blk in [(0, 6), (0, 7), (1, 2), (1, 3), (1, 4), (1, 5), (1, 6), (1, 7)]:
                extras_map[blk] = {3: [warm_mm], 9: [warm_mm]}
            g1 = qk_thunks(1)
            extras_map[(0, 0)] = {4 * i: [g1[i]] for i in range(4)}
            extras_map[(0, 1)] = {4 * i: [g1[i + 4]] for i in range(4)}
            g2 = qk_thunks(2)
            extras_map[(0, 2)] = {4 * i: [g2[i]] for i in range(4)}
            extras_map[(0, 3)] = {4 * i: [g2[i + 4]] for i in range(4)}
            g3 = qk_thunks(3)
            extras_map[(0, 4)] = {4 * i: [g3[i]] for i in range(4)}
            extras_map[(0, 5)] = {4 * i: [g3[i + 4]] for i in range(4)}
            # Wo chunks for qb=0 ride inside qb=1's first two j-loops (they
            # only need qb0's attn rows, all normalized by then).
            # (qb0,h7)'s deferred normalize sits at extras[6] of (1,0), so
            # qb0's Wo chunks must come after it.
            wo0 = [(0, mo, n) for mo in range(KT) for n in range(QW // 512)]
            slots = (
                [(0, 7), (0, 9), (0, 11), (0, 13), (0, 15),
                 (1, 2), (1, 5), (1, 8), (1, 11), (1, 14),
                 (2, 12), (2, 14), (3, 12), (3, 14), (4, 13), (5, 13)]
            )
            for args, (hh, jj) in zip(wo0, slots):
                extras_map.setdefault((1, hh), {}).setdefault(jj, []).append(
                    lambda a=args: wo_chunk(*a)
                )

            pending, tail = None, []
            for qb in range(NQB):
                for h in range(NHC):
                    extras = {
                        j: list(ths)
                        for j, ths in extras_map.get((qb, h), {}).items()
                    }
                    if pending is not None:
                        # Normalize the previous block a few iterations into
                        # this block's j-loop: by then its SBUF eviction and
                        # reciprocal are long done, so the broadcast matmuls
                        # never stall the PE stream.
                        blk = pending
                        extras.setdefault(6, [])
                        extras[6] = [lambda b=blk: normalize(b)] + extras[6]
                    pending, tail = attention_jloop(h, qb, extras, tail)
            for th in tail:
                th()
            normalize(pending)
            for args in [(1, mo, n) for mo in range(KT) for n in range(QW // 512)]:
                wo_chunk(*args)

    nc.compile()
    _CACHE["nc"] = nc
    return nc


def _prep_core_inputs(x, Wq, bq, Wk, bk, Wv, bv, Wo, bo):
    bf16 = ml_dtypes.bfloat16
    bo_eff = (bo + bv @ Wo.T).astype(np.float32)
    in_maps = []
    for c in range(8):
        b, g = c // 2, c % 2
        sl = slice(g * HG, (g + 1) * HG)
        m = {
            "xT": np.ascontiguousarray(x[b].T).astype(bf16),
            "wq": np.ascontiguousarray(Wq[sl, :].T).astype(bf16),
            "wk": np.ascontiguousarray(Wk[sl, :].T).astype(bf16),
            "wv": np.ascontiguousarray(Wv[sl, :].T).astype(bf16),
            "wo": np.ascontiguousarray(Wo[:, sl].T).astype(bf16),
            "bq": np.ascontiguousarray(
                bq[sl].astype(np.float32).reshape(CT, 128).T
            ),
            "bk": np.ascontiguousarray(
                bk[sl].astype(np.float32).reshape(CT, 128).T
            ),
            "bo": np.ascontiguousarray(bo_eff.reshape(KT, 128).T)
            if g == 0
            else np.zeros((128, KT), np.float32),
        }
        in_maps.append(m)
    return in_maps


def kernel(x, Wq, bq, Wk, bk, Wv, bv, Wo, bo):
    global LAST_RESULT
    x = np.asarray(x, np.float32)
    nc = _build()
    in_maps = _prep_core_inputs(
        x,
        np.asarray(Wq, np.float32),
        np.asarray(bq, np.float32),
        np.asarray(Wk, np.float32),
        np.asarray(bk, np.float32),
        np.asarray(Wv, np.float32),
        np.asarray(bv, np.float32),
        np.asarray(Wo, np.float32),
        np.asarray(bo, np.float32),
    )
    res = run_bass_kernel_spmd(nc, in_maps, list(range(8)))
    LAST_RESULT = res
    out = np.empty((B, S, H), np.float32)
    for b in range(B):
        yT = res.results[2 * b]["yT"] + res.results[2 * b + 1]["yT"]
        out[b] = yT.T
    return out
